# revision 6
# baseline (speedup 1.0000x reference)
"""Distributed Trainium2 Bass kernel for the additive-attention module.

Strategy (8 NeuronCores, data-parallel over batch):
  - 32 batch elements, 8 cores -> 4 sequences per core.
  - Sequences are ranked by source_length (descending); core c gets ranks
    {j*8+c : j=0..3} so that "slot" j has similar lengths on every core
    (one SPMD graph; per-slot trip counts are compile-time).
  - Host packs, per core, the valid rows enc[:len_b, b, :] of its 4
    sequences into a contiguous [R, H] buffer (slot j padded to the slot
    maximum, a multiple of 128) AND its transpose [H, R] — the transposed
    copy feeds the energy matmul (contraction over H needs H on the
    partition dim), the natural copy feeds the context matmul.
  - On device (per core, fp16 compute, fp32 accumulate):
      energyT[h, s] = sum_k W2T[k, h] * encT[k, s]        (PE, fp16)
      pre[h, j]     = W1 @ hidden[j] + attn_b             (PE, fp16)
      tanhE         = tanh(energyT + pre[:, j])           (ACT, fused bias)
      scores[j, s]  = sum_h v[h] * tanhE[h, s]            (PE, fp16)
      masked softmax over s (runtime lengths)             (DVE+ACT)
      context[j, :] = sum_s attnw[j, s] * enc[s, :]       (PE, fp16)
  - Host scatters the per-core [4, S] attention weights and [4, H]
    contexts back to the full (B, S) / (B, H) outputs.
"""

import numpy as np

import concourse.bass as bass
import concourse.mybir as mybir
import concourse.tile as tile
from concourse import bacc, bass_utils
from concourse.masks import make_identity

S, B, H = 2048, 32, 1024
NCORES = 8
NSLOT = B // NCORES  # 4
KB = H // 128  # 8 k-blocks
HB = H // 128  # 8 h-blocks
CHUNK = 512

F16 = mybir.dt.float16
F32 = mybir.dt.float32

_GRAPH_CACHE = {}
LAST_RESULT = None  # BassKernelResults of the most recent run (for profiling)


def _build(slot_lens):
    """Build + compile the SPMD graph for the given per-slot padded lengths."""
    offs = [0]
    for L in slot_lens:
        offs.append(offs[-1] + L)
    R = offs[-1]

    nc = bacc.Bacc("TRN2", target_bir_lowering=False, debug=False, num_devices=NCORES)

    enc_t = nc.dram_tensor("enc_t", [H, R], F32, kind="ExternalInput").ap()
    enc_n = nc.dram_tensor("enc_n", [R, H], F32, kind="ExternalInput").ap()
    hid = nc.dram_tensor("hid", [NSLOT, H], F32, kind="ExternalInput").ap()
    len_f = nc.dram_tensor("len_f", [NSLOT, 1], F32, kind="ExternalInput").ap()
    w1t = nc.dram_tensor("w1t", [H, H], F32, kind="ExternalInput").ap()
    w2t = nc.dram_tensor("w2t", [H, H], F32, kind="ExternalInput").ap()
    bias_d = nc.dram_tensor("bias", [H], F32, kind="ExternalInput").ap()
    v_d = nc.dram_tensor("v", [H], F32, kind="ExternalInput").ap()
    iota_d = nc.dram_tensor("iota", [NSLOT, S], F32, kind="ExternalInput").ap()
    attnw_out = nc.dram_tensor("attnw_out", [NSLOT, S], F32, kind="ExternalOutput").ap()
    ctx_out = nc.dram_tensor("ctx_out", [NSLOT, H], F32, kind="ExternalOutput").ap()

    # chunk list: (slot j, local col start, width)
    chunks = []
    for j, L in enumerate(slot_lens):
        c0 = 0
        while c0 < L:
            w = min(CHUNK, L - c0)
            chunks.append((j, c0, w))
            c0 += w

    with tile.TileContext(nc) as tc:
        with (
            tc.tile_pool(name="consts", bufs=1) as consts,
            tc.tile_pool(name="xtp", bufs=3) as xtp,
            tc.tile_pool(name="etp", bufs=2) as etp,
            tc.tile_pool(name="encp", bufs=3) as encp,
            tc.tile_pool(name="pe_ps", bufs=2, space="PSUM") as pe_ps,
            tc.tile_pool(name="s_ps", bufs=2, space="PSUM") as s_ps,
            tc.tile_pool(name="c_ps", bufs=2, space="PSUM") as c_ps,
            tc.tile_pool(name="t_ps", bufs=1, space="PSUM") as t_ps,
        ):
            # ---- constants / weights ----
            w2t_f16 = consts.tile([128, KB, H], F16)
            for kb in range(KB):
                nc.gpsimd.dma_start(
                    out=w2t_f16[:, kb, :], in_=w2t[kb * 128 : (kb + 1) * 128, :]
                )
            w1t_f16 = consts.tile([128, KB, H], F16)
            for kb in range(KB):
                nc.gpsimd.dma_start(
                    out=w1t_f16[:, kb, :], in_=w1t[kb * 128 : (kb + 1) * 128, :]
                )
            v_f16 = consts.tile([128, HB], F16)
            nc.gpsimd.dma_start(out=v_f16, in_=v_d.rearrange("(hb p) -> p hb", p=128))
            bias_sb = consts.tile([128, HB], F32)
            nc.sync.dma_start(out=bias_sb, in_=bias_d.rearrange("(hb p) -> p hb", p=128))
            iota_sb = consts.tile([1, S], F32)
            nc.sync.dma_start(out=iota_sb, in_=iota_d[0:1, :])
            len_sb = consts.tile([1, NSLOT], F32)
            nc.sync.dma_start(out=len_sb, in_=len_f.rearrange("j one -> one j"))
            ident = consts.tile([128, 128], F16)
            make_identity(nc, ident)

            hid_sb = consts.tile([NSLOT, H], F32)
            nc.sync.dma_start(out=hid_sb, in_=hid)
            hid_f16 = consts.tile([NSLOT, H], F16)
            nc.vector.tensor_copy(hid_f16, hid_sb)

            # hidT[k, j] via PE transpose of [NSLOT, 128] blocks
            hidT_f16 = consts.tile([128, KB, NSLOT], F16)
            for kb in range(KB):
                pt = t_ps.tile([128, NSLOT], F16, tag="tp16")
                nc.tensor.transpose(
                    pt, hid_f16[:, kb * 128 : (kb + 1) * 128], ident[:NSLOT, :NSLOT]
                )
                nc.vector.tensor_copy(hidT_f16[:, kb, :], pt)

            # preT[h, j] = sum_k W1T[k, h] hidT[k, j]  (+ attn_b)
            preT_sb = consts.tile([128, HB, NSLOT], F32)
            for hb in range(HB):
                pp = t_ps.tile([128, NSLOT], F32)
                for kb in range(KB):
                    nc.tensor.matmul(
                        pp,
                        w1t_f16[:, kb, hb * 128 : (hb + 1) * 128],
                        hidT_f16[:, kb, :],
                        start=(kb == 0),
                        stop=(kb == KB - 1),
                    )
                nc.vector.tensor_scalar_add(preT_sb[:, hb, :], pp, bias_sb[:, hb : hb + 1])

            # ---- energy + scores ----
            scores_t = [consts.tile([1, S], F32, tag=f"scores{j}", name=f"scores{j}") for j in range(NSLOT)]
            for j in range(NSLOT):
                nc.vector.memset(scores_t[j], 0.0)

            def emit_vdot(et_tile, j, c0, w):
                ps = s_ps.tile([1, CHUNK], F32)
                for hb in range(HB):
                    nc.tensor.matmul(
                        ps[:, :w],
                        v_f16[:, hb : hb + 1],
                        et_tile[:, hb, :w],
                        start=(hb == 0),
                        stop=(hb == HB - 1),
                    )
                nc.vector.tensor_copy(scores_t[j][:, c0 : c0 + w], ps[:, :w])

            prev = None
            for j, c0, w in chunks:
                xt = xtp.tile([128, KB, CHUNK], F16)
                for kb in range(KB):
                    nc.gpsimd.dma_start(
                        out=xt[:, kb, :w],
                        in_=enc_t[kb * 128 : (kb + 1) * 128, offs[j] + c0 : offs[j] + c0 + w],
                    )
                et = etp.tile([128, HB, CHUNK], F16)
                for hb in range(HB):
                    pe = pe_ps.tile([128, CHUNK], F32)
                    for kb in range(KB):
                        nc.tensor.matmul(
                            pe[:, :w],
                            w2t_f16[:, kb, hb * 128 : (hb + 1) * 128],
                            xt[:, kb, :w],
                            start=(kb == 0),
                            stop=(kb == KB - 1),
                        )
                    nc.scalar.activation(
                        et[:, hb, :w],
                        pe[:, :w],
                        mybir.ActivationFunctionType.Tanh,
                        bias=preT_sb[:, hb, j : j + 1],
                    )
                if prev is not None:
                    emit_vdot(*prev)
                prev = (et, j, c0, w)
            emit_vdot(*prev)

            # ---- masked softmax over s (runtime lengths), per slot ----
            attnw_f16 = consts.tile([NSLOT, S], F16)
            for j in range(NSLOT):
                penal = consts.tile([1, S], F32, tag="penal")
                nc.vector.tensor_scalar(
                    penal,
                    iota_sb,
                    len_sb[:, j : j + 1],
                    -1e30,
                    op0=mybir.AluOpType.is_ge,
                    op1=mybir.AluOpType.mult,
                )
                nc.vector.tensor_add(scores_t[j], scores_t[j], penal)
                negmax = consts.tile([1, 1], F32, tag="negmax")
                nc.vector.tensor_reduce(
                    negmax, scores_t[j], axis=mybir.AxisListType.X,
                    op=mybir.AluOpType.max, negate=True,
                )
                aw_j = consts.tile([1, S], F32, tag=f"aw{j}")
                sumexp = consts.tile([1, 1], F32, tag="sumexp")
                nc.scalar.activation(
                    aw_j,
                    scores_t[j],
                    mybir.ActivationFunctionType.Exp,
                    bias=negmax,
                    accum_out=sumexp,
                )
                rsum = consts.tile([1, 1], F32, tag="rsum")
                nc.vector.reciprocal(rsum, sumexp)
                nc.vector.tensor_scalar_mul(aw_j, aw_j, rsum)
                nc.sync.dma_start(out=attnw_out[j : j + 1, :], in_=aw_j)
                # assemble fp16 row j (cast + partition remap via SWDGE DMA)
                nc.gpsimd.dma_start(out=attnw_f16[j : j + 1, :], in_=aw_j)
            nblk = R // 128
            awT = consts.tile([128, nblk, NSLOT], F16)
            for j, L in enumerate(slot_lens):
                for t in range(L // 128):
                    blk = (offs[j] + t * 128) // 128
                    pt = t_ps.tile([128, NSLOT], F16, tag="tp16")
                    nc.tensor.transpose(
                        pt,
                        attnw_f16[:, t * 128 : (t + 1) * 128],
                        ident[:NSLOT, :NSLOT],
                    )
                    nc.vector.tensor_copy(awT[:, blk, :], pt[:, :])

            # ---- context ----
            for j, L in enumerate(slot_lens):
                nt = L // 128
                pc = c_ps.tile([33, CHUNK], F32)
                for t in range(nt):
                    blk = (offs[j] + t * 128) // 128
                    en = encp.tile([128, H], F16)
                    nc.gpsimd.dma_start(
                        out=en, in_=enc_n[blk * 128 : (blk + 1) * 128, :]
                    )
                    for half in range(2):
                        nc.tensor.matmul(
                            pc[half * 32 : half * 32 + 1, :],
                            awT[:, blk, j : j + 1],
                            en[:, half * CHUNK : (half + 1) * CHUNK],
                            start=(t == 0),
                            stop=(t == nt - 1),
                        )
                ctx_j = consts.tile([1, H], F32, tag=f"ctx{j}")
                for half in range(2):
                    nc.vector.tensor_copy(
                        ctx_j[:, half * CHUNK : (half + 1) * CHUNK],
                        pc[half * 32 : half * 32 + 1, :],
                    )
                nc.sync.dma_start(out=ctx_out[j : j + 1, :], in_=ctx_j)

    nc.compile()
    return nc


def _pad128(n):
    return max(128, (int(n) + 127) // 128 * 128)


def kernel(hidden, encoder_outputs, source_lengths, attn_w, attn_b, v):
    global LAST_RESULT
    hidden = np.asarray(hidden, dtype=np.float32)
    encoder_outputs = np.asarray(encoder_outputs, dtype=np.float32)
    lens = np.asarray(source_lengths).astype(np.int64)
    attn_w = np.asarray(attn_w, dtype=np.float32)
    attn_b = np.asarray(attn_b, dtype=np.float32)
    v = np.asarray(v, dtype=np.float32)

    # rank sequences by length desc; slot j of core c gets rank j*NCORES+c
    order = np.argsort(-lens, kind="stable")
    assign = order.reshape(NSLOT, NCORES)  # [slot, core] -> batch index
    slot_lens = tuple(_pad128(lens[assign[j]].max()) for j in range(NSLOT))
    offs = np.concatenate([[0], np.cumsum(slot_lens)])
    R = int(offs[-1])

    key = slot_lens
    if key not in _GRAPH_CACHE:
        _GRAPH_CACHE[key] = _build(slot_lens)
    nc = _GRAPH_CACHE[key]

    # weights in [k, h] layout (contraction dim on partitions)
    w1t = np.ascontiguousarray(attn_w[:, :H].T)
    w2t = np.ascontiguousarray(attn_w[:, H:].T)
    iota = np.broadcast_to(np.arange(S, dtype=np.float32), (NSLOT, S)).copy()

    in_maps = []
    for c in range(NCORES):
        bs = assign[:, c]
        enc_n = np.zeros((R, H), dtype=np.float32)
        for j in range(NSLOT):
            b = int(bs[j])
            L = int(lens[b])
            enc_n[offs[j] : offs[j] + L] = encoder_outputs[:L, b, :]
        enc_t = np.ascontiguousarray(enc_n.T)
        in_maps.append(
            {
                "enc_t": enc_t,
                "enc_n": enc_n,
                "hid": np.ascontiguousarray(hidden[bs]),
                "len_f": lens[bs].astype(np.float32).reshape(NSLOT, 1),
                "w1t": w1t,
                "w2t": w2t,
                "bias": attn_b,
                "v": v,
                "iota": iota,
            }
        )

    res = bass_utils.run_bass_kernel_spmd(nc, in_maps, core_ids=list(range(NCORES)))
    LAST_RESULT = res

    context = np.zeros((B, H), dtype=np.float32)
    attention_weights = np.zeros((B, S), dtype=np.float32)
    for c in range(NCORES):
        for j in range(NSLOT):
            b = int(assign[j, c])
            context[b] = res.results[c]["ctx_out"][j]
            attention_weights[b] = res.results[c]["attnw_out"][j]
    return context, attention_weights
